# revision 1
# baseline (speedup 1.0000x reference)
"""CosineClassifier Trainium2 kernel.

pred[b, c] = (img[b]/||img[b]||) . (concept[b,c]/||concept[b,c]||) / TEMP

Sharding: batch (128) split across 8 cores, 16 samples/core, no comms.

Per-core plan (memory-bound: 201 MB of concept per core):
  - concept tiles stream in natural layout [class->partition, emb->free]
  - DVE tensor_tensor_reduce computes dot(concept_row, img) in ONE pass
  - ACT activation(Square, accum_out) computes ||concept_row||^2 in ONE pass
  - epilogue: rsqrt via ACT-sqrt seed + 2 Newton steps (fp32-exact),
    PE transpose to get classes contiguous for the output DMA.
"""
import sys

for _p in ('/opt/trn_rl_repo',):
    if _p not in sys.path:
        sys.path.insert(0, _p)

import numpy as np

BS, NCLS, D = 128, 4096, 768
NCORES = 8
BPC = BS // NCORES          # samples per core
P = 128
CHUNKS = NCLS // P          # 32 class-chunks of 128 per sample
TCH = 4                     # class-chunks per DMA (512 classes = 1.5 MB)
NMAC = CHUNKS // TCH
TEMP = 0.05
INV_TEMP = 1.0 / TEMP

_CACHE = {}


def _split_multiwaits(nc, mybir):
    """This toolchain's CoreV3 codegen accepts at most 1 sync-wait per
    instruction (2 for EventSemaphore); Tile sometimes attaches more.
    Move extras onto single-wait NOPs inserted just before, same engine."""
    n = 0
    for f in nc.m.functions:
        for bb in f.blocks:
            il = bb.instructions
            if not any(
                i.sync_info is not None and i.sync_info.on_wait
                and len(i.sync_info.on_wait) > 1 for i in il
            ):
                continue
            out = []
            for inst in il:
                si = inst.sync_info
                cap = 2 if isinstance(inst, mybir.InstEventSemaphore) else 1
                if si is not None and si.on_wait and len(si.on_wait) > cap:
                    waits = list(si.on_wait)
                    for k, w in enumerate(waits[cap:]):
                        out.append(mybir.InstNoOp(
                            name=f"{inst.name}-wsplit{k}",
                            engine=inst.engine,
                            sync_info=mybir.SyncInfo(on_wait=[w], on_update=[]),
                            bass_nofuse=True,
                        ))
                        n += 1
                    si.on_wait = waits[:cap]
                out.append(inst)
            bb.instructions = out
    return n


def _build():
    from concourse import bass, mybir, tile, masks

    f32 = mybir.dt.float32
    Alu = mybir.AluOpType
    Act = mybir.ActivationFunctionType

    nc = bass.Bass("TRN2", target_bir_lowering=False, debug=False, num_devices=1)
    img = nc.dram_tensor("img", [BPC, D], f32, kind="ExternalInput").ap()
    concept = nc.dram_tensor(
        "concept", [BPC, NCLS, D], f32, kind="ExternalInput").ap()
    pred = nc.dram_tensor("pred", [BPC, NCLS], f32, kind="ExternalOutput").ap()

    with tile.TileContext(nc) as tc:
        with (
            tc.tile_pool(name="big", bufs=4) as big_pool,
            tc.tile_pool(name="imgp", bufs=2) as img_pool,
            tc.tile_pool(name="scr", bufs=2) as scr_pool,
            tc.tile_pool(name="res", bufs=1) as res_pool,
            tc.tile_pool(name="epi", bufs=2) as epi_pool,
            tc.tile_pool(name="psum", bufs=2,
                         space=bass.MemorySpace.PSUM) as psum_pool,
        ):
            y_all = res_pool.tile([P, BPC * CHUNKS], f32)   # dots
            s_all = res_pool.tile([P, BPC * CHUNKS], f32)   # |c|^2 (ACT)
            # |c|^2 for the DVE-offloaded chunks (separate tile so ACT and
            # DVE never write the same tile; merged before the epilogue)
            NOFF = 3                                        # chunks/sample on DVE
            s2_all = res_pool.tile([P, BPC * NOFF], f32)
            si_all = res_pool.tile([P, BPC], f32)           # |img|^2
            identity = res_pool.tile([P, P], f32)
            masks.make_identity(nc, identity[:])

            for b in range(BPC):
                imgb = img_pool.tile([P, D], f32, tag="imgb")
                nc.sync.dma_start(imgb[:], img[b:b + 1, :].to_broadcast((P, D)))
                si_scr = scr_pool.tile([P, D], f32, tag="sqscr")
                nc.scalar.activation(
                    si_scr[:], imgb[:], Act.Square,
                    accum_out=si_all[:, b:b + 1])
                for m in range(NMAC):
                    big = big_pool.tile([P, TCH * D], f32, tag="big")
                    src = concept[b, m * TCH * P:(m + 1) * TCH * P, :] \
                        .rearrange("(t p) d -> p t d", p=P)
                    dst = big[:].rearrange("p (t d) -> p t d", t=TCH)
                    nc.sync.dma_start(dst, src)
                    for t in range(TCH):
                        col = b * CHUNKS + m * TCH + t
                        cslice = big[:, t * D:(t + 1) * D]
                        ttr_scr = scr_pool.tile([P, D], f32, tag="ttrscr")
                        nc.vector.scalar_tensor_tensor(
                            out=ttr_scr[:],
                            in0=cslice, scalar=1.0, in1=imgb[:],
                            op0=Alu.mult, op1=Alu.mult,
                            accum_out=y_all[:, col:col + 1])
                        g = m * TCH + t
                        if g >= CHUNKS - NOFF:
                            # ACT is the bottleneck engine: offload the last
                            # 3 square-sums per sample to DVE
                            sq2_scr = scr_pool.tile([P, D], f32, tag="sq2scr")
                            c2 = b * NOFF + (g - (CHUNKS - NOFF))
                            nc.vector.scalar_tensor_tensor(
                                out=sq2_scr[:], in0=cslice, scalar=1.0,
                                in1=cslice, op0=Alu.mult, op1=Alu.mult,
                                accum_out=s2_all[:, c2:c2 + 1])
                        else:
                            sq_scr = scr_pool.tile([P, D], f32, tag="sqscr")
                            nc.scalar.activation(
                                sq_scr[:], cslice, Act.Square,
                                accum_out=s_all[:, col:col + 1])

            # merge the DVE-computed square-sums into s_all's column layout
            for b in range(BPC):
                nc.vector.tensor_copy(
                    s_all[:, b * CHUNKS + CHUNKS - NOFF:(b + 1) * CHUNKS],
                    s2_all[:, b * NOFF:(b + 1) * NOFF])

            # epilogue: pred = y * rsqrt(s*si) / TEMP, classes -> contiguous
            for b in range(BPC):
                sb = s_all[:, b * CHUNKS:(b + 1) * CHUNKS]
                yb = y_all[:, b * CHUNKS:(b + 1) * CHUNKS]
                q = epi_pool.tile([P, CHUNKS], f32, tag="q")
                nc.vector.tensor_scalar_mul(q[:], sb, si_all[:, b:b + 1])
                nc.vector.tensor_scalar_max(q[:], q[:], 1e-38)
                r = epi_pool.tile([P, CHUNKS], f32, tag="r")
                nc.scalar.activation(r[:], q[:], Act.Sqrt)
                nc.vector.reciprocal(r[:], r[:])
                t1 = epi_pool.tile([P, CHUNKS], f32, tag="t1")
                for _ in range(2):  # Newton: r <- r*(1.5 - 0.5*q*r^2)
                    nc.vector.tensor_mul(t1[:], r[:], r[:])
                    nc.vector.tensor_mul(t1[:], t1[:], q[:])
                    nc.vector.tensor_scalar(
                        out=t1[:], in0=t1[:], scalar1=-0.5, scalar2=1.5,
                        op0=Alu.mult, op1=Alu.add)
                    nc.vector.tensor_mul(r[:], r[:], t1[:])
                pb = epi_pool.tile([P, CHUNKS], f32, tag="pb")
                nc.vector.tensor_mul(pb[:], yb, r[:])
                nc.vector.tensor_scalar_mul(pb[:], pb[:], INV_TEMP)
                pt = psum_pool.tile([CHUNKS, P], f32, tag="pt")
                nc.tensor.transpose(pt[:], pb[:], identity[:])
                po = epi_pool.tile([CHUNKS, P], f32, tag="po")
                nc.vector.tensor_copy(po[:], pt[:])
                nc.sync.dma_start(
                    pred[b].rearrange("(g f) -> g f", f=P), po[:])

    _split_multiwaits(nc, mybir)
    return nc


def _get_nc():
    if 'nc' not in _CACHE:
        _CACHE['nc'] = _build()
    return _CACHE['nc']


def kernel(img: np.ndarray, concept: np.ndarray, **run_kwargs) -> np.ndarray:
    from concourse import bass_utils

    img = np.ascontiguousarray(img, dtype=np.float32)
    concept = np.ascontiguousarray(concept, dtype=np.float32)
    assert img.shape == (BS, D) and concept.shape == (BS, NCLS, D)

    nc = _get_nc()
    in_maps = [
        {"img": img[i * BPC:(i + 1) * BPC],
         "concept": concept[i * BPC:(i + 1) * BPC]}
        for i in range(NCORES)
    ]
    res = bass_utils.run_bass_kernel_spmd(
        nc, in_maps, core_ids=list(range(NCORES)), **run_kwargs)
    out = np.concatenate([r["pred"] for r in res.results], axis=0)
    if run_kwargs:
        _CACHE['last_results'] = res
    return out



# revision 8
# speedup vs baseline: 1.0877x; 1.0877x over previous
"""CosineClassifier Trainium2 kernel.

pred[b, c] = (img[b]/||img[b]||) . (concept[b,c]/||concept[b,c]||) / TEMP

Sharding: batch (128) split across 8 cores, 16 samples/core, no comms.

Per-core plan (memory-bound: 201 MB of concept per core):
  - concept tiles stream in natural layout [class->partition, emb->free];
    the DMA subsystem sustains ~400 GB/s when not back-pressured by
    compute, so compute is balanced to stay just under the DMA rate:
      dots    -> DVE scalar_tensor_tensor + accum_out  (32/sample, ~0.95us)
      squares -> ACT activation(Square) + accum_out    (32/sample, ~0.93us)
  - img row broadcast to 128 partitions via PE matmul (ones[1,128]^T @
    img_row) instead of a 393 KB broadcast DMA per sample; saves 6.3 MB
    of DMA traffic per core. PSUM->SBUF drain split ACT/DVE.
  - 1/(TEMP*||img_b||) precomputed once for all 16 samples, PE-broadcast
    into a [128,16] scalar table, folded into the epilogue multiply.
  - epilogue: rinv = 1/sqrt(||c||^2) via ACT-sqrt + exact DVE reciprocal
    (fp32-accurate), pb = y * wtab_b * rinv in ONE fused DVE op, PE
    transpose for a contiguous output DMA.
"""
import sys

for _p in ('/opt/trn_rl_repo',):
    if _p not in sys.path:
        sys.path.insert(0, _p)

import numpy as np

BS, NCLS, D = 128, 4096, 768
NCORES = 8
BPC = BS // NCORES          # samples per core
P = 128
CHUNKS = NCLS // P          # 32 class-chunks of 128 per sample
TCH = 4                     # class-chunks per DMA (512 classes = 1.5 MB)
NMAC = CHUNKS // TCH
TEMP = 0.05
INV_TEMP = 1.0 / TEMP

BIG_BUFS = 6

_CACHE = {}


def _split_multiwaits(nc, mybir):
    """This toolchain's CoreV3 codegen accepts at most 1 sync-wait per
    instruction (2 for EventSemaphore); Tile sometimes attaches more.
    Move extras onto single-wait NOPs inserted just before, same engine."""
    n = 0
    for f in nc.m.functions:
        for bb in f.blocks:
            il = bb.instructions
            if not any(
                i.sync_info is not None and i.sync_info.on_wait
                and len(i.sync_info.on_wait) > 1 for i in il
            ):
                continue
            out = []
            for inst in il:
                si = inst.sync_info
                cap = 2 if isinstance(inst, mybir.InstEventSemaphore) else 1
                if si is not None and si.on_wait and len(si.on_wait) > cap:
                    waits = list(si.on_wait)
                    for k, w in enumerate(waits[cap:]):
                        out.append(mybir.InstNoOp(
                            name=f"{inst.name}-wsplit{k}",
                            engine=inst.engine,
                            sync_info=mybir.SyncInfo(on_wait=[w], on_update=[]),
                            bass_nofuse=True,
                        ))
                        n += 1
                    si.on_wait = waits[:cap]
                out.append(inst)
            bb.instructions = out
    return n


def _build():
    from concourse import bass, mybir, tile, masks

    f32 = mybir.dt.float32
    Alu = mybir.AluOpType
    Act = mybir.ActivationFunctionType

    nc = bass.Bass("TRN2", target_bir_lowering=False, debug=False, num_devices=1)
    img = nc.dram_tensor("img", [BPC, D], f32, kind="ExternalInput").ap()
    concept = nc.dram_tensor(
        "concept", [BPC, NCLS, D], f32, kind="ExternalInput").ap()
    pred = nc.dram_tensor("pred", [BPC, NCLS], f32, kind="ExternalOutput").ap()

    with tile.TileContext(nc) as tc:
        with (
            tc.tile_pool(name="big", bufs=BIG_BUFS) as big_pool,
            tc.tile_pool(name="imgp", bufs=3) as img_pool,
            tc.tile_pool(name="scrv", bufs=2) as scrv_pool,   # DVE scratch
            tc.tile_pool(name="scra", bufs=2) as scra_pool,   # ACT scratch
            tc.tile_pool(name="res", bufs=1) as res_pool,
            tc.tile_pool(name="epi", bufs=2) as epi_pool,
            tc.tile_pool(name="psb", bufs=2,
                         space=bass.MemorySpace.PSUM) as psb_pool,   # img bcast
            tc.tile_pool(name="pst", bufs=2,
                         space=bass.MemorySpace.PSUM) as pst_pool,   # transposes
            tc.tile_pool(name="ps1", bufs=1,
                         space=bass.MemorySpace.PSUM) as ps1_pool,   # one-shot
        ):
            y_all = res_pool.tile([P, BPC * CHUNKS], f32)   # dots (DVE)
            s_all = res_pool.tile([P, BPC * CHUNKS], f32)   # |c|^2 (ACT)
            identity = res_pool.tile([P, P], f32)
            masks.make_identity(nc, identity[:])
            ones1 = res_pool.tile([1, P], f32)
            nc.gpsimd.memset(ones1[:], 1.0)

            # ---- img: one 48 KB load + per-sample scale table -------------
            img_all = res_pool.tile([BPC, D], f32)
            nc.sync.dma_start(img_all[:], img[:, :])
            # flat copy on partition 0: PE moving operand must be based at
            # partition 0/32/64, so slice broadcast inputs from here
            img_flat = res_pool.tile([1, BPC * D], f32)
            nc.sync.dma_start(
                img_flat[:],
                img[:, :].rearrange("b d -> (b d)").rearrange(
                    "(x f) -> x f", x=1))
            # wtab[:, b] = 1 / (TEMP * ||img_b||) on all 128 partitions
            sia = res_pool.tile([BPC, 1], f32)
            sia_scr = res_pool.tile([BPC, D], f32)
            nc.scalar.activation(sia_scr[:], img_all[:], Act.Square,
                                 accum_out=sia[:])
            sqa = res_pool.tile([BPC, 1], f32)
            # sqrt(sia * TEMP^2) = TEMP * ||img_b||
            nc.scalar.activation(sqa[:], sia[:], Act.Sqrt, scale=TEMP * TEMP)
            rqa = res_pool.tile([BPC, 1], f32)
            nc.vector.reciprocal(rqa[:], sqa[:])
            rqa_t = ps1_pool.tile([1, BPC], f32, tag="rqat")
            nc.tensor.transpose(rqa_t[:], rqa[:], identity[:BPC, :BPC])
            rqa_sb = res_pool.tile([1, BPC], f32)
            nc.vector.tensor_copy(rqa_sb[:], rqa_t[:])
            wtab_ps = ps1_pool.tile([P, BPC], f32, tag="wtab")
            nc.tensor.matmul(wtab_ps[:], ones1[:], rqa_sb[:])
            wtab = res_pool.tile([P, BPC], f32)
            nc.vector.tensor_copy(wtab[:], wtab_ps[:])

            HALF = D // 2

            def emit_img_bcast(b):
                """PE-broadcast img row b to [128, D] in SBUF."""
                p0 = psb_pool.tile([P, HALF], f32, tag="p0")
                p1 = psb_pool.tile([P, HALF], f32, tag="p1")
                nc.tensor.matmul(
                    p0[:], ones1[:], img_flat[:, b * D:b * D + HALF])
                nc.tensor.matmul(
                    p1[:], ones1[:], img_flat[:, b * D + HALF:(b + 1) * D])
                imgb = img_pool.tile([P, D], f32, tag="imgb")
                nc.scalar.activation(imgb[:, :HALF], p0[:], Act.Copy)
                nc.vector.tensor_copy(imgb[:, HALF:], p1[:])
                return imgb

            imgb_next = emit_img_bcast(0)

            for b in range(BPC):
                imgb = imgb_next
                if b + 1 < BPC:
                    imgb_next = emit_img_bcast(b + 1)

                for m in range(NMAC):
                    big = big_pool.tile([P, TCH * D], f32, tag="big")
                    src = concept[b, m * TCH * P:(m + 1) * TCH * P, :] \
                        .rearrange("(t p) d -> p t d", p=P)
                    dst = big[:].rearrange("p (t d) -> p t d", t=TCH)
                    nc.sync.dma_start(dst, src)

                    for t in range(TCH):
                        g = m * TCH + t
                        col = b * CHUNKS + g
                        cslice = big[:, t * D:(t + 1) * D]
                        scr = scrv_pool.tile([P, D], f32, tag="vd")
                        nc.vector.scalar_tensor_tensor(
                            out=scr[:], in0=cslice, scalar=1.0,
                            in1=imgb[:], op0=Alu.mult, op1=Alu.mult,
                            accum_out=y_all[:, col:col + 1])
                        scr2 = scra_pool.tile([P, D], f32, tag="as")
                        nc.scalar.activation(
                            scr2[:], cslice, Act.Square,
                            accum_out=s_all[:, col:col + 1])

                # ---- epilogue: pred[b] = y * wtab_b / sqrt(s) -------------
                c0 = b * CHUNKS
                r = epi_pool.tile([P, CHUNKS], f32, tag="r")
                nc.scalar.activation(
                    r[:], s_all[:, c0:c0 + CHUNKS], Act.Sqrt)
                rinv = epi_pool.tile([P, CHUNKS], f32, tag="rinv")
                nc.vector.reciprocal(rinv[:], r[:])
                pb = epi_pool.tile([P, CHUNKS], f32, tag="pb")
                nc.vector.scalar_tensor_tensor(
                    out=pb[:], in0=y_all[:, c0:c0 + CHUNKS],
                    scalar=wtab[:, b:b + 1], in1=rinv[:],
                    op0=Alu.mult, op1=Alu.mult)
                pt = pst_pool.tile([CHUNKS, P], f32, tag="pt")
                nc.tensor.transpose(pt[:], pb[:], identity[:])
                po = epi_pool.tile([CHUNKS, P], f32, tag="po")
                nc.scalar.activation(po[:], pt[:], Act.Copy)
                nc.sync.dma_start(
                    pred[b].rearrange("(g f) -> g f", f=P), po[:])

    _split_multiwaits(nc, mybir)
    return nc


def _get_nc():
    if 'nc' not in _CACHE:
        _CACHE['nc'] = _build()
    return _CACHE['nc']


def kernel(img: np.ndarray, concept: np.ndarray, **run_kwargs) -> np.ndarray:
    from concourse import bass_utils

    img = np.ascontiguousarray(img, dtype=np.float32)
    concept = np.ascontiguousarray(concept, dtype=np.float32)
    assert img.shape == (BS, D) and concept.shape == (BS, NCLS, D)

    nc = _get_nc()
    in_maps = [
        {"img": img[i * BPC:(i + 1) * BPC],
         "concept": concept[i * BPC:(i + 1) * BPC]}
        for i in range(NCORES)
    ]
    res = bass_utils.run_bass_kernel_spmd(
        nc, in_maps, core_ids=list(range(NCORES)), **run_kwargs)
    out = np.concatenate([r["pred"] for r in res.results], axis=0)
    if run_kwargs:
        _CACHE['last_results'] = res
    return out


# revision 11
# speedup vs baseline: 1.0967x; 1.0083x over previous
"""CosineClassifier Trainium2 kernel.

pred[b, c] = (img[b]/||img[b]||) . (concept[b,c]/||concept[b,c]||) / TEMP

Sharding: batch (128) split across 8 cores, 16 samples/core, no comms.

Per-core plan (memory-bound: 201 MB of concept per core):
  - concept tiles stream in natural layout [class->partition, emb->free];
    the DMA subsystem sustains ~400 GB/s when not back-pressured by
    compute, so compute is balanced to stay just under the DMA rate:
      dots    -> DVE scalar_tensor_tensor + accum_out  (32/sample, ~0.95us)
      squares -> ACT activation(Square) + accum_out    (32/sample, ~0.93us)
  - img row broadcast to 128 partitions via PE matmul (ones[1,128]^T @
    img_row) instead of a 393 KB broadcast DMA per sample; saves 6.3 MB
    of DMA traffic per core. PSUM->SBUF drain split ACT/DVE.
  - 1/(TEMP*||img_b||) precomputed once for all 16 samples, PE-broadcast
    into a [128,16] scalar table, folded into the epilogue multiply.
  - epilogue: rinv = 1/sqrt(||c||^2) via ACT-sqrt + exact DVE reciprocal
    (fp32-accurate), pb = y * wtab_b * rinv in ONE fused DVE op, PE
    transpose for a contiguous output DMA.
"""
import sys

for _p in ('/opt/trn_rl_repo',):
    if _p not in sys.path:
        sys.path.insert(0, _p)

import numpy as np

BS, NCLS, D = 128, 4096, 768
NCORES = 8
BPC = BS // NCORES          # samples per core
P = 128
CHUNKS = NCLS // P          # 32 class-chunks of 128 per sample
TCH = 4                     # class-chunks per DMA (512 classes = 1.5 MB)
NMAC = CHUNKS // TCH
TEMP = 0.05
INV_TEMP = 1.0 / TEMP

BIG_BUFS = 8
N_SQ_DVE = 2                # trailing chunks/sample whose square runs on DVE
EPI_AT_M = 3                # emit sample b-1's epilogue after this DMA of b

_CACHE = {}


def _split_multiwaits(nc, mybir):
    """This toolchain's CoreV3 codegen accepts at most 1 sync-wait per
    instruction (2 for EventSemaphore); Tile sometimes attaches more.
    Move extras onto single-wait NOPs inserted just before, same engine."""
    n = 0
    for f in nc.m.functions:
        for bb in f.blocks:
            il = bb.instructions
            if not any(
                i.sync_info is not None and i.sync_info.on_wait
                and len(i.sync_info.on_wait) > 1 for i in il
            ):
                continue
            out = []
            for inst in il:
                si = inst.sync_info
                cap = 2 if isinstance(inst, mybir.InstEventSemaphore) else 1
                if si is not None and si.on_wait and len(si.on_wait) > cap:
                    waits = list(si.on_wait)
                    for k, w in enumerate(waits[cap:]):
                        out.append(mybir.InstNoOp(
                            name=f"{inst.name}-wsplit{k}",
                            engine=inst.engine,
                            sync_info=mybir.SyncInfo(on_wait=[w], on_update=[]),
                            bass_nofuse=True,
                        ))
                        n += 1
                    si.on_wait = waits[:cap]
                out.append(inst)
            bb.instructions = out
    return n


def _build():
    from concourse import bass, mybir, tile, masks

    f32 = mybir.dt.float32
    Alu = mybir.AluOpType
    Act = mybir.ActivationFunctionType

    nc = bass.Bass("TRN2", target_bir_lowering=False, debug=False, num_devices=1)
    img = nc.dram_tensor("img", [BPC, D], f32, kind="ExternalInput").ap()
    concept = nc.dram_tensor(
        "concept", [BPC, NCLS, D], f32, kind="ExternalInput").ap()
    pred = nc.dram_tensor("pred", [BPC, NCLS], f32, kind="ExternalOutput").ap()

    with tile.TileContext(nc) as tc:
        with (
            tc.tile_pool(name="big", bufs=BIG_BUFS) as big_pool,
            tc.tile_pool(name="imgp", bufs=3) as img_pool,
            tc.tile_pool(name="scrv", bufs=2) as scrv_pool,   # DVE scratch
            tc.tile_pool(name="scra", bufs=2) as scra_pool,   # ACT scratch
            tc.tile_pool(name="res", bufs=1) as res_pool,
            tc.tile_pool(name="epi", bufs=2) as epi_pool,
            tc.tile_pool(name="psb", bufs=2,
                         space=bass.MemorySpace.PSUM) as psb_pool,   # img bcast
            tc.tile_pool(name="pst", bufs=2,
                         space=bass.MemorySpace.PSUM) as pst_pool,   # transposes
            tc.tile_pool(name="ps1", bufs=1,
                         space=bass.MemorySpace.PSUM) as ps1_pool,   # one-shot
        ):
            y_all = res_pool.tile([P, BPC * CHUNKS], f32)   # dots (DVE)
            NSA = CHUNKS - N_SQ_DVE
            s_all = res_pool.tile([P, BPC * NSA], f32)      # |c|^2 (ACT)
            s_dve = res_pool.tile([P, BPC * N_SQ_DVE], f32)  # |c|^2 (DVE)
            identity = res_pool.tile([P, P], f32)
            masks.make_identity(nc, identity[:])
            ones1 = res_pool.tile([1, P], f32)
            nc.gpsimd.memset(ones1[:], 1.0)

            # ---- img: one 48 KB load + per-sample scale table -------------
            img_all = res_pool.tile([BPC, D], f32)
            nc.sync.dma_start(img_all[:], img[:, :])
            # flat copy on partition 0: PE moving operand must be based at
            # partition 0/32/64, so slice broadcast inputs from here
            img_flat = res_pool.tile([1, BPC * D], f32)
            nc.sync.dma_start(
                img_flat[:],
                img[:, :].rearrange("b d -> (b d)").rearrange(
                    "(x f) -> x f", x=1))
            # wtab[:, b] = 1 / (TEMP * ||img_b||) on all 128 partitions
            sia = res_pool.tile([BPC, 1], f32)
            sia_scr = res_pool.tile([BPC, D], f32)
            nc.scalar.activation(sia_scr[:], img_all[:], Act.Square,
                                 accum_out=sia[:])
            sqa = res_pool.tile([BPC, 1], f32)
            # sqrt(sia * TEMP^2) = TEMP * ||img_b||
            nc.scalar.activation(sqa[:], sia[:], Act.Sqrt, scale=TEMP * TEMP)
            rqa = res_pool.tile([BPC, 1], f32)
            nc.vector.reciprocal(rqa[:], sqa[:])
            rqa_t = ps1_pool.tile([1, BPC], f32, tag="rqat")
            nc.tensor.transpose(rqa_t[:], rqa[:], identity[:BPC, :BPC])
            rqa_sb = res_pool.tile([1, BPC], f32)
            nc.vector.tensor_copy(rqa_sb[:], rqa_t[:])
            wtab_ps = ps1_pool.tile([P, BPC], f32, tag="wtab")
            nc.tensor.matmul(wtab_ps[:], ones1[:], rqa_sb[:])
            wtab = res_pool.tile([P, BPC], f32)
            nc.vector.tensor_copy(wtab[:], wtab_ps[:])

            HALF = D // 2

            def emit_img_bcast(b):
                """PE-broadcast img row b to [128, D] in SBUF."""
                p0 = psb_pool.tile([P, HALF], f32, tag="p0")
                p1 = psb_pool.tile([P, HALF], f32, tag="p1")
                nc.tensor.matmul(
                    p0[:], ones1[:], img_flat[:, b * D:b * D + HALF])
                nc.tensor.matmul(
                    p1[:], ones1[:], img_flat[:, b * D + HALF:(b + 1) * D])
                imgb = img_pool.tile([P, D], f32, tag="imgb")
                nc.scalar.activation(imgb[:, :HALF], p0[:], Act.Copy)
                nc.vector.tensor_copy(imgb[:, HALF:], p1[:])
                return imgb

            imgb_next = emit_img_bcast(0)

            def emit_epilogue(b):
                """pred[b] = y * wtab_b / sqrt(s); emitted ~half a sample
                after b's accumulators complete so the cross-engine chain
                (ACT sqrt -> DVE recip/pb -> PE transpose -> DVE drain)
                never head-of-line-blocks the streaming ops."""
                c0 = b * CHUNKS
                sa = b * NSA
                sd = b * N_SQ_DVE
                r = epi_pool.tile([P, CHUNKS], f32, tag="r")
                nc.scalar.activation(
                    r[:, :NSA], s_all[:, sa:sa + NSA], Act.Sqrt)
                nc.scalar.activation(
                    r[:, NSA:], s_dve[:, sd:sd + N_SQ_DVE], Act.Sqrt)
                rinv = epi_pool.tile([P, CHUNKS], f32, tag="rinv")
                nc.vector.reciprocal(rinv[:], r[:])
                pb = epi_pool.tile([P, CHUNKS], f32, tag="pb")
                nc.vector.scalar_tensor_tensor(
                    out=pb[:], in0=y_all[:, c0:c0 + CHUNKS],
                    scalar=wtab[:, b:b + 1], in1=rinv[:],
                    op0=Alu.mult, op1=Alu.mult)
                pt = pst_pool.tile([CHUNKS, P], f32, tag="pt")
                nc.tensor.transpose(pt[:], pb[:], identity[:])
                po = epi_pool.tile([CHUNKS, P], f32, tag="po")
                nc.vector.tensor_copy(po[:], pt[:])
                nc.sync.dma_start(
                    pred[b].rearrange("(g f) -> g f", f=P), po[:])

            for b in range(BPC):
                imgb = imgb_next
                if b + 1 < BPC:
                    imgb_next = emit_img_bcast(b + 1)

                for m in range(NMAC):
                    big = big_pool.tile([P, TCH * D], f32, tag="big")
                    src = concept[b, m * TCH * P:(m + 1) * TCH * P, :] \
                        .rearrange("(t p) d -> p t d", p=P)
                    dst = big[:].rearrange("p (t d) -> p t d", t=TCH)
                    nc.sync.dma_start(dst, src)

                    for t in range(TCH):
                        g = m * TCH + t
                        cslice = big[:, t * D:(t + 1) * D]
                        scr = scrv_pool.tile([P, D], f32, tag="vd")
                        nc.vector.scalar_tensor_tensor(
                            out=scr[:], in0=cslice, scalar=1.0,
                            in1=imgb[:], op0=Alu.mult, op1=Alu.mult,
                            accum_out=y_all[:, b * CHUNKS + g:b * CHUNKS + g + 1])
                        if g < NSA:
                            col = b * NSA + g
                            scr2 = scra_pool.tile([P, D], f32, tag="as")
                            nc.scalar.activation(
                                scr2[:], cslice, Act.Square,
                                accum_out=s_all[:, col:col + 1])
                        else:
                            col = b * N_SQ_DVE + (g - NSA)
                            scr2 = scrv_pool.tile([P, D], f32, tag="vs")
                            nc.vector.scalar_tensor_tensor(
                                out=scr2[:], in0=cslice, scalar=1.0,
                                in1=cslice, op0=Alu.mult, op1=Alu.mult,
                                accum_out=s_dve[:, col:col + 1])

                    if m == EPI_AT_M and b > 0:
                        emit_epilogue(b - 1)

            emit_epilogue(BPC - 1)

    _split_multiwaits(nc, mybir)
    return nc


def _get_nc():
    if 'nc' not in _CACHE:
        _CACHE['nc'] = _build()
    return _CACHE['nc']


def kernel(img: np.ndarray, concept: np.ndarray, **run_kwargs) -> np.ndarray:
    from concourse import bass_utils

    img = np.ascontiguousarray(img, dtype=np.float32)
    concept = np.ascontiguousarray(concept, dtype=np.float32)
    assert img.shape == (BS, D) and concept.shape == (BS, NCLS, D)

    nc = _get_nc()
    in_maps = [
        {"img": img[i * BPC:(i + 1) * BPC],
         "concept": concept[i * BPC:(i + 1) * BPC]}
        for i in range(NCORES)
    ]
    res = bass_utils.run_bass_kernel_spmd(
        nc, in_maps, core_ids=list(range(NCORES)), **run_kwargs)
    out = np.concatenate([r["pred"] for r in res.results], axis=0)
    if run_kwargs:
        _CACHE['last_results'] = res
    return out


# revision 12
# speedup vs baseline: 1.1205x; 1.0217x over previous
"""CosineClassifier Trainium2 kernel.

pred[b, c] = (img[b]/||img[b]||) . (concept[b,c]/||concept[b,c]||) / TEMP

Sharding: batch (128) split across 8 cores, 16 samples/core, no comms.

Per-core plan (memory-bound: 201 MB of concept per core):
  - concept tiles stream in natural layout [class->partition, emb->free];
    the DMA subsystem sustains ~400 GB/s when not back-pressured by
    compute, so compute is balanced to stay just under the DMA rate:
      dots    -> DVE scalar_tensor_tensor + accum_out  (32/sample, ~0.95us)
      squares -> ACT activation(Square) + accum_out    (32/sample, ~0.93us)
  - img row broadcast to 128 partitions via PE matmul (ones[1,128]^T @
    img_row) instead of a 393 KB broadcast DMA per sample; saves 6.3 MB
    of DMA traffic per core. PSUM->SBUF drain split ACT/DVE.
  - 1/(TEMP*||img_b||) precomputed once for all 16 samples, PE-broadcast
    into a [128,16] scalar table, folded into the epilogue multiply.
  - epilogue: rinv = 1/sqrt(||c||^2) via ACT-sqrt + exact DVE reciprocal
    (fp32-accurate), pb = y * wtab_b * rinv in ONE fused DVE op, PE
    transpose for a contiguous output DMA.
"""
import sys

for _p in ('/opt/trn_rl_repo',):
    if _p not in sys.path:
        sys.path.insert(0, _p)

import numpy as np

BS, NCLS, D = 128, 4096, 768
NCORES = 8
BPC = BS // NCORES          # samples per core
P = 128
CHUNKS = NCLS // P          # 32 class-chunks of 128 per sample
TCH = 4                     # class-chunks per DMA (512 classes = 1.5 MB)
NMAC = CHUNKS // TCH
TEMP = 0.05
INV_TEMP = 1.0 / TEMP

BIG_BUFS = 8
N_SQ_DVE = 2                # trailing chunks/sample whose square runs on DVE
EPI_AT_M = 3                # emit sample b-1's epilogue after this DMA of b

_CACHE = {}


def _split_multiwaits(nc, mybir):
    """This toolchain's CoreV3 codegen accepts at most 1 sync-wait per
    instruction (2 for EventSemaphore); Tile sometimes attaches more.
    Move extras onto single-wait NOPs inserted just before, same engine."""
    n = 0
    for f in nc.m.functions:
        for bb in f.blocks:
            il = bb.instructions
            if not any(
                i.sync_info is not None and i.sync_info.on_wait
                and len(i.sync_info.on_wait) > 1 for i in il
            ):
                continue
            out = []
            for inst in il:
                si = inst.sync_info
                cap = 2 if isinstance(inst, mybir.InstEventSemaphore) else 1
                if si is not None and si.on_wait and len(si.on_wait) > cap:
                    waits = list(si.on_wait)
                    for k, w in enumerate(waits[cap:]):
                        out.append(mybir.InstNoOp(
                            name=f"{inst.name}-wsplit{k}",
                            engine=inst.engine,
                            sync_info=mybir.SyncInfo(on_wait=[w], on_update=[]),
                            bass_nofuse=True,
                        ))
                        n += 1
                    si.on_wait = waits[:cap]
                out.append(inst)
            bb.instructions = out
    return n


def _build():
    from concourse import bass, mybir, tile, masks

    f32 = mybir.dt.float32
    Alu = mybir.AluOpType
    Act = mybir.ActivationFunctionType

    nc = bass.Bass("TRN2", target_bir_lowering=False, debug=False, num_devices=1)
    img = nc.dram_tensor("img", [BPC, D], f32, kind="ExternalInput").ap()
    concept = nc.dram_tensor(
        "concept", [BPC, NCLS, D], f32, kind="ExternalInput").ap()
    pred = nc.dram_tensor("pred", [BPC, NCLS], f32, kind="ExternalOutput").ap()

    with tile.TileContext(nc) as tc:
        with (
            tc.tile_pool(name="big", bufs=BIG_BUFS) as big_pool,
            tc.tile_pool(name="imgp", bufs=3) as img_pool,
            tc.tile_pool(name="scrv", bufs=2) as scrv_pool,   # DVE scratch
            tc.tile_pool(name="scra", bufs=2) as scra_pool,   # ACT scratch
            tc.tile_pool(name="res", bufs=1) as res_pool,
            tc.tile_pool(name="epi", bufs=2) as epi_pool,
            tc.tile_pool(name="psb", bufs=2,
                         space=bass.MemorySpace.PSUM) as psb_pool,   # img bcast
            tc.tile_pool(name="pst", bufs=2,
                         space=bass.MemorySpace.PSUM) as pst_pool,   # transposes
            tc.tile_pool(name="ps1", bufs=1,
                         space=bass.MemorySpace.PSUM) as ps1_pool,   # one-shot
        ):
            y_all = res_pool.tile([P, BPC * CHUNKS], f32)   # dots (DVE)
            NSA = CHUNKS - N_SQ_DVE
            s_all = res_pool.tile([P, BPC * NSA], f32)      # |c|^2 (ACT)
            s_dve = res_pool.tile([P, BPC * N_SQ_DVE], f32)  # |c|^2 (DVE)
            identity = res_pool.tile([P, P], f32)
            masks.make_identity(nc, identity[:])
            ones1 = res_pool.tile([1, P], f32)
            nc.gpsimd.memset(ones1[:], 1.0)

            # ---- img: one 48 KB load + per-sample scale table -------------
            img_all = res_pool.tile([BPC, D], f32)
            nc.sync.dma_start(img_all[:], img[:, :])
            # flat copy on partition 0: PE moving operand must be based at
            # partition 0/32/64, so slice broadcast inputs from here
            img_flat = res_pool.tile([1, BPC * D], f32)
            nc.sync.dma_start(
                img_flat[:],
                img[:, :].rearrange("b d -> (b d)").rearrange(
                    "(x f) -> x f", x=1))
            # wtab[:, b] = 1 / (TEMP * ||img_b||) on all 128 partitions
            sia = res_pool.tile([BPC, 1], f32)
            sia_scr = res_pool.tile([BPC, D], f32)
            nc.scalar.activation(sia_scr[:], img_all[:], Act.Square,
                                 accum_out=sia[:])
            sqa = res_pool.tile([BPC, 1], f32)
            # sqrt(sia * TEMP^2) = TEMP * ||img_b||
            nc.scalar.activation(sqa[:], sia[:], Act.Sqrt, scale=TEMP * TEMP)
            rqa = res_pool.tile([BPC, 1], f32)
            nc.vector.reciprocal(rqa[:], sqa[:])
            rqa_t = ps1_pool.tile([1, BPC], f32, tag="rqat")
            nc.tensor.transpose(rqa_t[:], rqa[:], identity[:BPC, :BPC])
            rqa_sb = res_pool.tile([1, BPC], f32)
            nc.vector.tensor_copy(rqa_sb[:], rqa_t[:])
            wtab_ps = ps1_pool.tile([P, BPC], f32, tag="wtab")
            nc.tensor.matmul(wtab_ps[:], ones1[:], rqa_sb[:])
            wtab = res_pool.tile([P, BPC], f32)
            nc.vector.tensor_copy(wtab[:], wtab_ps[:])

            HALF = D // 2

            def emit_img_bcast(b):
                """PE-broadcast img row b to [128, D] in SBUF."""
                p0 = psb_pool.tile([P, HALF], f32, tag="p0")
                p1 = psb_pool.tile([P, HALF], f32, tag="p1")
                nc.tensor.matmul(
                    p0[:], ones1[:], img_flat[:, b * D:b * D + HALF])
                nc.tensor.matmul(
                    p1[:], ones1[:], img_flat[:, b * D + HALF:(b + 1) * D])
                imgb = img_pool.tile([P, D], f32, tag="imgb")
                nc.scalar.activation(imgb[:, :HALF], p0[:], Act.Copy)
                nc.vector.tensor_copy(imgb[:, HALF:], p1[:])
                return imgb

            imgb_next = emit_img_bcast(0)

            def emit_epilogue(b):
                """pred[b] = y * wtab_b / sqrt(s); emitted ~half a sample
                after b's accumulators complete so the cross-engine chain
                (ACT sqrt -> DVE recip/pb -> PE transpose -> DVE drain)
                never head-of-line-blocks the streaming ops."""
                c0 = b * CHUNKS
                sa = b * NSA
                sd = b * N_SQ_DVE
                r = epi_pool.tile([P, CHUNKS], f32, tag="r")
                nc.scalar.activation(
                    r[:, :NSA], s_all[:, sa:sa + NSA], Act.Sqrt)
                nc.scalar.activation(
                    r[:, NSA:], s_dve[:, sd:sd + N_SQ_DVE], Act.Sqrt)
                rinv = epi_pool.tile([P, CHUNKS], f32, tag="rinv")
                nc.vector.reciprocal(rinv[:], r[:])
                pb = epi_pool.tile([P, CHUNKS], f32, tag="pb")
                nc.vector.scalar_tensor_tensor(
                    out=pb[:], in0=y_all[:, c0:c0 + CHUNKS],
                    scalar=wtab[:, b:b + 1], in1=rinv[:],
                    op0=Alu.mult, op1=Alu.mult)
                pt = pst_pool.tile([CHUNKS, P], f32, tag="pt")
                nc.tensor.transpose(pt[:], pb[:], identity[:])
                po = epi_pool.tile([CHUNKS, P], f32, tag="po")
                nc.vector.tensor_copy(po[:], pt[:])
                nc.sync.dma_start(
                    pred[b].rearrange("(g f) -> g f", f=P), po[:])

            for b in range(BPC):
                imgb = imgb_next
                if b + 1 < BPC:
                    imgb_next = emit_img_bcast(b + 1)

                for m in range(NMAC):
                    big = big_pool.tile([P, TCH * D], f32, tag="big")
                    src = concept[b, m * TCH * P:(m + 1) * TCH * P, :] \
                        .rearrange("(t p) d -> p t d", p=P)
                    dst = big[:].rearrange("p (t d) -> p t d", t=TCH)
                    # alternate the two DMA issue rings (SP HWDGE / Pool
                    # SWDGE) so consecutive transfers overlap; a single
                    # ring executes its DMAs strictly FIFO
                    if m % 2 == 0:
                        nc.sync.dma_start(dst, src)
                    else:
                        nc.gpsimd.dma_start(dst, src)

                    for t in range(TCH):
                        g = m * TCH + t
                        cslice = big[:, t * D:(t + 1) * D]
                        scr = scrv_pool.tile([P, D], f32, tag="vd")
                        nc.vector.scalar_tensor_tensor(
                            out=scr[:], in0=cslice, scalar=1.0,
                            in1=imgb[:], op0=Alu.mult, op1=Alu.mult,
                            accum_out=y_all[:, b * CHUNKS + g:b * CHUNKS + g + 1])
                        if g < NSA:
                            col = b * NSA + g
                            scr2 = scra_pool.tile([P, D], f32, tag="as")
                            nc.scalar.activation(
                                scr2[:], cslice, Act.Square,
                                accum_out=s_all[:, col:col + 1])
                        else:
                            col = b * N_SQ_DVE + (g - NSA)
                            scr2 = scrv_pool.tile([P, D], f32, tag="vs")
                            nc.vector.scalar_tensor_tensor(
                                out=scr2[:], in0=cslice, scalar=1.0,
                                in1=cslice, op0=Alu.mult, op1=Alu.mult,
                                accum_out=s_dve[:, col:col + 1])

                    if m == EPI_AT_M and b > 0:
                        emit_epilogue(b - 1)

            emit_epilogue(BPC - 1)

    _split_multiwaits(nc, mybir)
    return nc


def _get_nc():
    if 'nc' not in _CACHE:
        _CACHE['nc'] = _build()
    return _CACHE['nc']


def kernel(img: np.ndarray, concept: np.ndarray, **run_kwargs) -> np.ndarray:
    from concourse import bass_utils

    img = np.ascontiguousarray(img, dtype=np.float32)
    concept = np.ascontiguousarray(concept, dtype=np.float32)
    assert img.shape == (BS, D) and concept.shape == (BS, NCLS, D)

    nc = _get_nc()
    in_maps = [
        {"img": img[i * BPC:(i + 1) * BPC],
         "concept": concept[i * BPC:(i + 1) * BPC]}
        for i in range(NCORES)
    ]
    res = bass_utils.run_bass_kernel_spmd(
        nc, in_maps, core_ids=list(range(NCORES)), **run_kwargs)
    out = np.concatenate([r["pred"] for r in res.results], axis=0)
    if run_kwargs:
        _CACHE['last_results'] = res
    return out
